# revision 17
# baseline (speedup 1.0000x reference)
"""Trainium2 Bass kernel for batched per-sample expert matmul (MoE routing).

Computes y[n, i] = relu(b[idxs[n], i] + sum_o w[idxs[n], i, o] * x[n, o])
for x (8192, 256), idxs (8192,), w (64, 256, 256), b (64, 256).

Strategy
--------
Host side (numpy, cheap):
  * Shard EXPERTS across cores, one expert per (core, ordinal) pair:
    rank experts by sample count and give ordinal o the experts ranked
    [8o, 8o+8), one per core.  Every core runs the same program over 8
    expert blocks (SPMD); block o is sized BS_o = roundup(max count in
    rank group o, 32), so the per-core tape is ~1.2k samples instead of
    a padded 2k.  Every expert's weights are sent exactly once:
    8 x 128 KB bf16 = 1 MB per core instead of the full 16 MB table.
  * Everything the PE touches is cast to bf16 on the host (w is a
    0.02-scale gaussian; bf16 keeps the absmax-relative error ~4e-3,
    well inside the tolerance) - this halves the HBM traffic.

Device side (one static Tile program, identical on all 8 cores).
W-stationary orientation: psum[i, n] = sum_p w[e, i, p-chunk] x[n, p-chunk]
puts the FEATURE dim on psum partitions and samples on the free dim, so
the per-sample bias b[e, i] becomes a per-partition scalar:

  for each expert block o, feature half f:
      psum_of[i, n] = sum over contraction chunks c of
                      wT[c*128+p, f*128+i] @ xT[c*128+p, n]   (2 matmuls)
      y_of = relu(psum_of + b[e, f*128+i])    one fused pass on DVE (f=0)
                                              or ACT (f=1), cast to bf16

  No bias matmuls (that saved 1/3 of the PE's streamed columns), and the
  16 psum tiles all fit in the 8 PSUM banks simultaneously - no release
  waits.  The lowered matmul puts the stationary-operand (w) DMA wait on
  LDWEIGHTS and the moving-operand (x) wait on MATMUL, so no instruction
  ever needs two semaphore waits.  The x/w stream rides the single SWDGE
  queue (gpsimd) in need order, alternating x-block and w-slot batches
  (HWDGE transfers starve when the SWDGE stream saturates the shared
  SDMA engines, so only the tiny bias table and the y writebacks use the
  sync HWDGE ring).

Host side: scatter block rows back to the original sample order.
Pathologically skewed routing (an expert with > 512 samples) falls back
to a sorted-shard multi-pass variant of the same program built on
128-sample single-expert segments.
"""

import os

import ml_dtypes
import numpy as np

import concourse.bacc as bacc
import concourse.bass as bass
import concourse.mybir as mybir
import concourse.tile as tile
from concourse.bass_utils import run_bass_kernel_spmd

N_CORES = 8
N_EXP = 64       # experts in the table
EPC = N_EXP // N_CORES  # experts per core
P = 128          # SBUF/PSUM partitions
F = 256          # feature dim (in_features == out_features == 256)
SEG = 128        # fallback-path segment size
MAX_NSEG = 32    # fallback-path per-pass segment budget
MAX_BS = 512     # psum-bank bound on samples per expert block

if os.environ.get("KBENCH_MM_DT", "bfloat16") == "bfloat16":
    MM_DT = mybir.dt.bfloat16
    NP_DT = ml_dtypes.bfloat16
else:
    MM_DT = mybir.dt.float32r
    NP_DT = np.float32


def _batches(n, sizes, rest):
    """Split range(n) into batches: explicit `sizes` first, then `rest`-sized."""
    out = []
    lo = 0
    i = 0
    while lo < n:
        sz = sizes[i] if i < len(sizes) else rest
        i += 1
        hi = min(n, lo + sz)
        out.append((lo, hi))
        lo = hi
    return out


# Set by the last kernel() call when KBENCH_TRACE=1 (used by test.py only).
LAST_EXEC_TIME_NS = None
LAST_TRACE = None


def _build_program(BS, compile=True):
    """One program: nb = len(BS) expert blocks, block i sized BS[i] samples.

    Weight slot i belongs to block i.  The w and x data for a batch of
    blocks are packed (host-side) into one contiguous column range of a
    single "stream" dram tensor, in need order:

      batch g (blocks [lo, hi)): [w slot lo .. w slot hi-1 | x c0 | x c1]
        w slot i: 512 cols, [(c*2+f)*128+fi] = w[e_i, f*128+fi, c*128+p]
        x chunk c: T_g cols,  xT[c*128+p, n]

    so each batch is ONE fully-contiguous DMA (one Q7 emission, one
    completion semaphore).  Batch 0 rides the sync HWDGE ring - it lands
    before the SWDGE flood saturates the SDMA engines, so the PE starts
    ~2us earlier; the rest stream through the SWDGE queue in FIFO need
    order.  Other dram tensors:
      bconst [128, 2*nb]   f32    bconst[p, i*2+f] = b[e_i, f*128+p]
      y      [128, 2, T]   bf16   y[p, f, off_i + n] = y_blk[n, f*128+p]
    """
    nb = len(BS)
    offs = [0]
    for s in BS:
        offs.append(offs[-1] + s)
    T = offs[-1]

    sbat = _stream_batches(nb)
    ybat = _batches(nb, [nb - 3], 3) if nb > 3 else [(0, nb)]
    scols = [(hi - lo) * 4 * P + 2 * (offs[hi] - offs[lo]) for lo, hi in sbat]
    soffs = [0]
    for sc in scols:
        soffs.append(soffs[-1] + sc)

    nc = bacc.Bacc(
        "TRN2", target_bir_lowering=False, debug=False, num_devices=N_CORES
    )
    s_d = nc.dram_tensor(
        "stream", [P, soffs[-1]], MM_DT, kind="ExternalInput"
    ).ap()
    b_d = nc.dram_tensor(
        "bconst", [P, 2 * nb], mybir.dt.float32, kind="ExternalInput"
    ).ap()
    y_d = nc.dram_tensor("y", [P, 2, T], MM_DT, kind="ExternalOutput").ap()

    f32 = mybir.dt.float32
    relu = mybir.ActivationFunctionType.Relu
    add = mybir.AluOpType.add
    amax = mybir.AluOpType.max

    # one psum tile per block holds both feature halves (f1 at a fixed
    # half-bank offset so each matmul output stays inside one bank);
    # PSUM allocation is bank-granular, so residency is counted in banks
    FS = [256 if s <= 256 else 512 for s in BS]
    banks = [2 * fs * 4 // 2048 for fs in FS]
    resident = sum(banks) <= 8

    with tile.TileContext(nc) as tc:
        with (
            tc.tile_pool(name="const", bufs=1) as const,
            tc.tile_pool(name="yout", bufs=1) as ypool,
            tc.tile_pool(
                name="ps", bufs=1 if resident else 8 // max(banks),
                space="PSUM",
            ) as pspool,
        ):
            # bias table first on the HWDGE ring (tiny)
            bc = const.tile([P, 2 * nb], f32, tag="bconst")
            nc.sync.dma_start(bc[:], b_d[:])

            sts = {}
            blk2bat = {}
            for g, (lo, hi) in enumerate(sbat):
                for i in range(lo, hi):
                    blk2bat[i] = g

            for g in range(len(sbat)):
                st = const.tile([P, scols[g]], MM_DT, tag=f"s{g}", name=f"s{g}")
                sts[g] = st
                nc.gpsimd.dma_start(st[:], s_d[:, soffs[g]:soffs[g + 1]])

            def wtile(i, c, f):
                g = blk2bat[i]
                lo, _ = sbat[g]
                base = ((i - lo) * 4 + c * 2 + f) * P
                return sts[g][:, base:base + P]

            def xchunk(i, c):
                g = blk2bat[i]
                lo, hi = sbat[g]
                bT = offs[hi] - offs[lo]
                base = (hi - lo) * 4 * P + c * bT + (offs[i] - offs[lo])
                return sts[g][:, base:base + BS[i]]

            yts = {}
            blk2y = {}
            for k, (lo, hi) in enumerate(ybat):
                for i in range(lo, hi):
                    blk2y[i] = k

            for i in range(nb):
                k = blk2y[i]
                ylo, yhi = ybat[k]
                ybT = offs[yhi] - offs[ylo]
                if i == ylo:
                    yts[k] = ypool.tile([P, 2 * ybT], MM_DT, tag=f"y{k}", name=f"yt{k}")
                ps = pspool.tile(
                    [P, 2 * FS[i]], f32, name=f"ps{i}",
                    **({"tag": f"ps{i}"} if resident else {}),
                )
                # f-major: each feature half's accumulation group must be
                # closed before the next one opens in the same PSUM bank
                # (one pending group per zero region)
                for f in (0, 1):
                    for c in (0, 1):
                        o = f * FS[i]
                        nc.tensor.matmul(
                            ps[:, o:o + BS[i]], wtile(i, c, f), xchunk(i, c),
                            start=(c == 0), stop=(c == 1),
                        )
                yb = offs[i] - offs[ylo]
                # fused relu(psum + bias[p]) with the f32->bf16 cast; the
                # two feature halves run on different engines in parallel
                nc.vector.tensor_scalar(
                    yts[k][:, yb:yb + BS[i]], ps[:, 0:BS[i]],
                    bc[:, 2 * i:2 * i + 1], 0.0, op0=add, op1=amax,
                )
                if i == nb - 1:
                    # the last block's f1 also goes on DVE: the ACT queue
                    # lags the PE by a few ops at the end, and the final y
                    # DMA must wait for this op
                    nc.vector.tensor_scalar(
                        yts[k][:, ybT + yb:ybT + yb + BS[i]],
                        ps[:, FS[i]:FS[i] + BS[i]],
                        bc[:, 2 * i + 1:2 * i + 2], 0.0, op0=add, op1=amax,
                    )
                else:
                    nc.scalar.activation(
                        yts[k][:, ybT + yb:ybT + yb + BS[i]],
                        ps[:, FS[i]:FS[i] + BS[i]],
                        relu, bias=bc[:, 2 * i + 1:2 * i + 2],
                    )
                if i == yhi - 1:
                    nc.sync.dma_start(
                        y_d[:, :, offs[ylo]:offs[yhi]],
                        yts[k][:].rearrange("p (f n) -> p f n", f=2),
                    )
    if compile:
        nc.compile()
    return nc


def _stream_batches(nb):
    """Stream batch plan over blocks: 1-block head batches (first data
    lands early), 2-block middle, 1-block tail (a short PE tail after
    the stream ends)."""
    return _batches(nb, [1, 1], 2)


def _prep_tables(w, b):
    # wprep[e, p, (c*2+f)*128+fi] = w[e, f*128+fi, c*128+p]
    wprep = np.ascontiguousarray(
        w.reshape(N_EXP, 2, P, 2, P)   # (e, f, fi, c, p)
        .transpose(0, 4, 3, 1, 2)      # (e, p, c, f, fi)
        .reshape(N_EXP, P, 4 * P)
        .astype(NP_DT)
    )
    # bprep[e, p, f] = b[e, f*128+p]
    bprep = np.ascontiguousarray(
        b.reshape(N_EXP, 2, P).transpose(0, 2, 1).astype(np.float32)
    )
    return wprep, bprep


def _run(nc, in_maps, trace):
    global LAST_EXEC_TIME_NS, LAST_TRACE
    res = run_bass_kernel_spmd(
        nc, in_maps, core_ids=list(range(N_CORES)), trace=trace
    )
    LAST_EXEC_TIME_NS = res.exec_time_ns
    LAST_TRACE = res.instructions_and_trace
    return res


def _pack_core(x, wprep, bprep, blocks, BS, offs, T):
    """blocks: per block (expert_id, sample_rows)."""
    xpad = np.zeros((T, F), dtype=np.float32)
    eids = np.empty(len(BS), dtype=np.int64)
    for i, (e, rows) in enumerate(blocks):
        eids[i] = e
        if len(rows):
            xpad[offs[i]:offs[i] + len(rows)] = x[rows]
    # xt[p, c, t] = xpad[t, c*128+p]
    xt = xpad.T.reshape(2, P, T).transpose(1, 0, 2).astype(NP_DT)
    wseg = wprep[eids]  # (nb, P, 512)
    parts = []
    for lo, hi in _stream_batches(len(BS)):
        parts.append(wseg[lo:hi].transpose(1, 0, 2).reshape(P, -1))
        parts.append(xt[:, :, offs[lo]:offs[hi]].reshape(P, -1))
    stream = np.ascontiguousarray(np.concatenate(parts, axis=1))
    bconst = np.ascontiguousarray(
        bprep[eids].transpose(1, 0, 2).reshape(P, 2 * len(BS))
    )
    return {"stream": stream, "bconst": bconst}


def _unpack_core(y, yraw, blocks, offs, T):
    # yraw [P, 2, T] -> y_blk[t, f*128+p]
    yfull = np.ascontiguousarray(
        yraw.astype(np.float32).transpose(1, 0, 2).reshape(F, T).T
    )
    for i, (e, rows) in enumerate(blocks):
        if len(rows):
            y[rows] = yfull[offs[i]:offs[i] + len(rows)]


def kernel(x: np.ndarray, idxs: np.ndarray, w: np.ndarray, b: np.ndarray) -> np.ndarray:
    x = np.ascontiguousarray(x, dtype=np.float32)
    w = np.ascontiguousarray(w, dtype=np.float32)
    b = np.ascontiguousarray(b, dtype=np.float32)
    idxs_np = np.asarray(idxs).astype(np.int64)

    B = x.shape[0]
    counts = np.bincount(idxs_np, minlength=N_EXP)
    wprep, bprep = _prep_tables(w, b)
    trace = bool(os.environ.get("KBENCH_TRACE"))
    y = np.empty((B, F), dtype=np.float32)

    order = np.argsort(idxs_np, kind="stable")
    sidx = idxs_np[order]
    estart = np.searchsorted(sidx, np.arange(N_EXP + 1))
    erows = [order[estart[e]:estart[e + 1]] for e in range(N_EXP)]

    if counts.max() <= MAX_BS:
        # Expert-sharded path: rank experts by count; ordinal o takes the
        # experts ranked [8o, 8o+8), one per core, with a shared block
        # size that covers the largest of them.
        rank = np.argsort(-counts, kind="stable")
        gmax = [
            int(counts[rank[o * N_CORES:(o + 1) * N_CORES]].max())
            for o in range(EPC)
        ]
        # ordinals descending in size; put the smallest block FIRST (its
        # batch is the first DMA the PE waits on) and the second-smallest
        # LAST (short PE/relu/writeback tail after the stream ends)
        perm = [EPC - 1] + list(range(EPC - 2)) + [EPC - 2]
        rank = np.concatenate(
            [rank[o * N_CORES:(o + 1) * N_CORES] for o in perm]
        )
        BS = [max(16, -(-gmax[o] // 16) * 16) for o in perm]
        offs = np.concatenate([[0], np.cumsum(BS)])
        T = int(offs[-1])
        nc = _build_program(BS)

        in_maps = []
        core_blocks = []
        for c in range(N_CORES):
            blocks = [
                (int(rank[o * N_CORES + c]), erows[rank[o * N_CORES + c]])
                for o in range(EPC)
            ]
            core_blocks.append(blocks)
            in_maps.append(_pack_core(x, wprep, bprep, blocks, BS, offs, T))

        res = _run(nc, in_maps, trace)
        for c in range(N_CORES):
            _unpack_core(y, res.results[c]["y"], core_blocks[c], offs, T)
        return y

    # Fallback for pathological skew: sorted-shard into 128-sample
    # single-expert segments (one weight slot per segment), multi-pass.
    S = B // N_CORES
    per_core = []
    for c in range(N_CORES):
        e = sidx[c * S:(c + 1) * S]
        sel = order[c * S:(c + 1) * S]
        segs = []
        i = 0
        while i < S:
            j = i
            while j < S and e[j] == e[i]:
                j += 1
            k = i
            while k < j:
                cnt = min(SEG, j - k)
                segs.append((int(e[i]), sel[k:k + cnt]))
                k += cnt
            i = j
        per_core.append(segs)

    npass = max(1, (max(len(s) for s in per_core) + MAX_NSEG - 1) // MAX_NSEG)
    nseg = MAX_NSEG if npass > 1 else max(2, max(len(s) for s in per_core))
    BS = [SEG] * nseg
    offs = np.concatenate([[0], np.cumsum(BS)])
    T = int(offs[-1])
    nc = _build_program(BS)

    for pi in range(npass):
        in_maps = []
        pass_blocks = []
        for c in range(N_CORES):
            segs = per_core[c][pi * MAX_NSEG:(pi + 1) * MAX_NSEG]
            blocks = [
                segs[s] if s < len(segs) else (0, order[0:0])
                for s in range(nseg)
            ]
            pass_blocks.append(blocks)
            in_maps.append(_pack_core(x, wprep, bprep, blocks, BS, offs, T))
        res = _run(nc, in_maps, trace)
        for c in range(N_CORES):
            _unpack_core(y, res.results[c]["y"], pass_blocks[c], offs, T)
    return y


# revision 18
# speedup vs baseline: 1.1988x; 1.1988x over previous
"""Trainium2 Bass kernel for batched per-sample expert matmul (MoE routing).

Computes y[n, i] = relu(b[idxs[n], i] + sum_o w[idxs[n], i, o] * x[n, o])
for x (8192, 256), idxs (8192,), w (64, 256, 256), b (64, 256).

Strategy
--------
Host side (numpy, cheap):
  * Shard EXPERTS across cores, one expert per (core, ordinal) pair:
    rank experts by sample count and give ordinal o the experts ranked
    [8o, 8o+8), one per core.  Every core runs the same program over 8
    expert blocks (SPMD); block o is sized BS_o = roundup(max count in
    rank group o, 32), so the per-core tape is ~1.2k samples instead of
    a padded 2k.  Every expert's weights are sent exactly once:
    8 x 128 KB bf16 = 1 MB per core instead of the full 16 MB table.
  * Everything the PE touches is cast to bf16 on the host (w is a
    0.02-scale gaussian; bf16 keeps the absmax-relative error ~4e-3,
    well inside the tolerance) - this halves the HBM traffic.

Device side (one static Tile program, identical on all 8 cores).
W-stationary orientation: psum[i, n] = sum_p w[e, i, p-chunk] x[n, p-chunk]
puts the FEATURE dim on psum partitions and samples on the free dim, so
the per-sample bias b[e, i] becomes a per-partition scalar:

  for each expert block o, feature half f:
      psum_of[i, n] = sum over contraction chunks c of
                      wT[c*128+p, f*128+i] @ xT[c*128+p, n]   (2 matmuls)
      y_of = relu(psum_of + b[e, f*128+i])    one fused pass on DVE (f=0)
                                              or ACT (f=1), cast to bf16

  No bias matmuls (that saved 1/3 of the PE's streamed columns), and the
  16 psum tiles all fit in the 8 PSUM banks simultaneously - no release
  waits.  The lowered matmul puts the stationary-operand (w) DMA wait on
  LDWEIGHTS and the moving-operand (x) wait on MATMUL, so no instruction
  ever needs two semaphore waits.  The x/w stream rides the single SWDGE
  queue (gpsimd) in need order, alternating x-block and w-slot batches
  (HWDGE transfers starve when the SWDGE stream saturates the shared
  SDMA engines, so only the tiny bias table and the y writebacks use the
  sync HWDGE ring).

Host side: scatter block rows back to the original sample order.
Pathologically skewed routing (an expert with > 512 samples) falls back
to a sorted-shard multi-pass variant of the same program built on
128-sample single-expert segments.
"""

import os

import ml_dtypes
import numpy as np

import concourse.bacc as bacc
import concourse.bass as bass
import concourse.mybir as mybir
import concourse.tile as tile
from concourse.bass_utils import run_bass_kernel_spmd

N_CORES = 8
N_EXP = 64       # experts in the table
EPC = N_EXP // N_CORES  # experts per core
P = 128          # SBUF/PSUM partitions
F = 256          # feature dim (in_features == out_features == 256)
SEG = 128        # fallback-path segment size
MAX_NSEG = 32    # fallback-path per-pass segment budget
MAX_BS = 512     # psum-bank bound on samples per expert block

if os.environ.get("KBENCH_MM_DT", "bfloat16") == "bfloat16":
    MM_DT = mybir.dt.bfloat16
    NP_DT = ml_dtypes.bfloat16
else:
    MM_DT = mybir.dt.float32r
    NP_DT = np.float32


def _batches(n, sizes, rest):
    """Split range(n) into batches: explicit `sizes` first, then `rest`-sized."""
    out = []
    lo = 0
    i = 0
    while lo < n:
        sz = sizes[i] if i < len(sizes) else rest
        i += 1
        hi = min(n, lo + sz)
        out.append((lo, hi))
        lo = hi
    return out


# Set by the last kernel() call when KBENCH_TRACE=1 (used by test.py only).
LAST_EXEC_TIME_NS = None
LAST_TRACE = None


def _build_program(BS, compile=True):
    """One program: nb = len(BS) expert blocks, block i sized BS[i] samples.

    Weight slot i belongs to block i.  The w and x data for a batch of
    blocks are packed (host-side) into one contiguous column range of a
    single "stream" dram tensor, in need order:

      batch g (blocks [lo, hi)): [w slot lo .. w slot hi-1 | x c0 | x c1]
        w slot i: 512 cols, [(c*2+f)*128+fi] = w[e_i, f*128+fi, c*128+p]
        x chunk c: T_g cols,  xT[c*128+p, n]

    so each batch is ONE fully-contiguous DMA (one Q7 emission, one
    completion semaphore).  Batch 0 rides the sync HWDGE ring - it lands
    before the SWDGE flood saturates the SDMA engines, so the PE starts
    ~2us earlier; the rest stream through the SWDGE queue in FIFO need
    order.  Other dram tensors:
      bconst [128, 2*nb]   f32    bconst[p, i*2+f] = b[e_i, f*128+p]
      y      [128, 2, T]   bf16   y[p, f, off_i + n] = y_blk[n, f*128+p]
    """
    nb = len(BS)
    offs = [0]
    for s in BS:
        offs.append(offs[-1] + s)
    T = offs[-1]

    sbat = _stream_batches(nb)
    ybat = _batches(nb, [3], 2)
    scols = [(hi - lo) * 4 * P + 2 * (offs[hi] - offs[lo]) for lo, hi in sbat]
    soffs = [0]
    for sc in scols:
        soffs.append(soffs[-1] + sc)

    nc = bacc.Bacc(
        "TRN2", target_bir_lowering=False, debug=False, num_devices=N_CORES
    )
    s_d = nc.dram_tensor(
        "stream", [P, soffs[-1]], MM_DT, kind="ExternalInput"
    ).ap()
    b_d = nc.dram_tensor(
        "bconst", [P, 2 * nb], mybir.dt.float32, kind="ExternalInput"
    ).ap()
    y_d = nc.dram_tensor("y", [P, 2, T], MM_DT, kind="ExternalOutput").ap()

    f32 = mybir.dt.float32
    relu = mybir.ActivationFunctionType.Relu
    add = mybir.AluOpType.add
    amax = mybir.AluOpType.max

    # one psum tile per block holds both feature halves (f1 at a fixed
    # half-bank offset so each matmul output stays inside one bank);
    # PSUM allocation is bank-granular, so residency is counted in banks
    FS = [256 if s <= 256 else 512 for s in BS]
    banks = [2 * fs * 4 // 2048 for fs in FS]
    resident = sum(banks) <= 8

    with tile.TileContext(nc) as tc:
        with (
            tc.tile_pool(name="const", bufs=1) as const,
            tc.tile_pool(name="yout", bufs=1) as ypool,
            tc.tile_pool(
                name="ps", bufs=1 if resident else 8 // max(banks),
                space="PSUM",
            ) as pspool,
        ):
            # bias table first on the HWDGE ring (tiny)
            bc = const.tile([P, 2 * nb], f32, tag="bconst")
            nc.sync.dma_start(bc[:], b_d[:])

            sts = {}
            blk2bat = {}
            for g, (lo, hi) in enumerate(sbat):
                for i in range(lo, hi):
                    blk2bat[i] = g

            for g in range(len(sbat)):
                st = const.tile([P, scols[g]], MM_DT, tag=f"s{g}", name=f"s{g}")
                sts[g] = st
                nc.gpsimd.dma_start(st[:], s_d[:, soffs[g]:soffs[g + 1]])

            def wtile(i, c, f):
                g = blk2bat[i]
                lo, _ = sbat[g]
                base = ((i - lo) * 4 + c * 2 + f) * P
                return sts[g][:, base:base + P]

            def xchunk(i, c):
                g = blk2bat[i]
                lo, hi = sbat[g]
                bT = offs[hi] - offs[lo]
                base = (hi - lo) * 4 * P + c * bT + (offs[i] - offs[lo])
                return sts[g][:, base:base + BS[i]]

            yts = {}
            blk2y = {}
            for k, (lo, hi) in enumerate(ybat):
                for i in range(lo, hi):
                    blk2y[i] = k

            for i in range(nb):
                k = blk2y[i]
                ylo, yhi = ybat[k]
                ybT = offs[yhi] - offs[ylo]
                if i == ylo:
                    yts[k] = ypool.tile([P, 2 * ybT], MM_DT, tag=f"y{k}", name=f"yt{k}")
                ps = pspool.tile(
                    [P, 2 * FS[i]], f32, name=f"ps{i}",
                    **({"tag": f"ps{i}"} if resident else {}),
                )
                # f-major: each feature half's accumulation group must be
                # closed before the next one opens in the same PSUM bank
                # (one pending group per zero region)
                for f in (0, 1):
                    for c in (0, 1):
                        o = f * FS[i]
                        nc.tensor.matmul(
                            ps[:, o:o + BS[i]], wtile(i, c, f), xchunk(i, c),
                            start=(c == 0), stop=(c == 1),
                        )
                yb = offs[i] - offs[ylo]
                # fused relu(psum + bias[p]) with the f32->bf16 cast; the
                # two feature halves run on different engines in parallel
                nc.vector.tensor_scalar(
                    yts[k][:, yb:yb + BS[i]], ps[:, 0:BS[i]],
                    bc[:, 2 * i:2 * i + 1], 0.0, op0=add, op1=amax,
                )
                if i == nb - 1:
                    # the last block's f1 also goes on DVE: the ACT queue
                    # lags the PE by a few ops at the end, and the final y
                    # DMA must wait for this op
                    nc.vector.tensor_scalar(
                        yts[k][:, ybT + yb:ybT + yb + BS[i]],
                        ps[:, FS[i]:FS[i] + BS[i]],
                        bc[:, 2 * i + 1:2 * i + 2], 0.0, op0=add, op1=amax,
                    )
                else:
                    nc.scalar.activation(
                        yts[k][:, ybT + yb:ybT + yb + BS[i]],
                        ps[:, FS[i]:FS[i] + BS[i]],
                        relu, bias=bc[:, 2 * i + 1:2 * i + 2],
                    )
                if i == yhi - 1:
                    nc.sync.dma_start(
                        y_d[:, :, offs[ylo]:offs[yhi]],
                        yts[k][:].rearrange("p (f n) -> p f n", f=2),
                    )
    if compile:
        nc.compile()
    return nc


def _stream_batches(nb):
    """Stream batch plan over blocks: 1-block head batches (first data
    lands early), 2-block middle, 1-block tail (a short PE tail after
    the stream ends)."""
    return _batches(nb, [1, 1], 2)


def _prep_tables(w, b):
    # wprep[e, p, (c*2+f)*128+fi] = w[e, f*128+fi, c*128+p]
    wprep = np.ascontiguousarray(
        w.reshape(N_EXP, 2, P, 2, P)   # (e, f, fi, c, p)
        .transpose(0, 4, 3, 1, 2)      # (e, p, c, f, fi)
        .reshape(N_EXP, P, 4 * P)
        .astype(NP_DT)
    )
    # bprep[e, p, f] = b[e, f*128+p]
    bprep = np.ascontiguousarray(
        b.reshape(N_EXP, 2, P).transpose(0, 2, 1).astype(np.float32)
    )
    return wprep, bprep


def _run(nc, in_maps, trace):
    global LAST_EXEC_TIME_NS, LAST_TRACE
    res = run_bass_kernel_spmd(
        nc, in_maps, core_ids=list(range(N_CORES)), trace=trace
    )
    LAST_EXEC_TIME_NS = res.exec_time_ns
    LAST_TRACE = res.instructions_and_trace
    return res


def _pack_core(x, wprep, bprep, blocks, BS, offs, T):
    """blocks: per block (expert_id, sample_rows)."""
    xpad = np.zeros((T, F), dtype=np.float32)
    eids = np.empty(len(BS), dtype=np.int64)
    for i, (e, rows) in enumerate(blocks):
        eids[i] = e
        if len(rows):
            xpad[offs[i]:offs[i] + len(rows)] = x[rows]
    # xt[p, c, t] = xpad[t, c*128+p]
    xt = xpad.T.reshape(2, P, T).transpose(1, 0, 2).astype(NP_DT)
    wseg = wprep[eids]  # (nb, P, 512)
    parts = []
    for lo, hi in _stream_batches(len(BS)):
        parts.append(wseg[lo:hi].transpose(1, 0, 2).reshape(P, -1))
        parts.append(xt[:, :, offs[lo]:offs[hi]].reshape(P, -1))
    stream = np.ascontiguousarray(np.concatenate(parts, axis=1))
    bconst = np.ascontiguousarray(
        bprep[eids].transpose(1, 0, 2).reshape(P, 2 * len(BS))
    )
    return {"stream": stream, "bconst": bconst}


def _unpack_core(y, yraw, blocks, offs, T):
    # yraw [P, 2, T] -> y_blk[t, f*128+p]
    yfull = np.ascontiguousarray(
        yraw.astype(np.float32).transpose(1, 0, 2).reshape(F, T).T
    )
    for i, (e, rows) in enumerate(blocks):
        if len(rows):
            y[rows] = yfull[offs[i]:offs[i] + len(rows)]


def kernel(x: np.ndarray, idxs: np.ndarray, w: np.ndarray, b: np.ndarray) -> np.ndarray:
    x = np.ascontiguousarray(x, dtype=np.float32)
    w = np.ascontiguousarray(w, dtype=np.float32)
    b = np.ascontiguousarray(b, dtype=np.float32)
    idxs_np = np.asarray(idxs).astype(np.int64)

    B = x.shape[0]
    counts = np.bincount(idxs_np, minlength=N_EXP)
    wprep, bprep = _prep_tables(w, b)
    trace = bool(os.environ.get("KBENCH_TRACE"))
    y = np.empty((B, F), dtype=np.float32)

    order = np.argsort(idxs_np, kind="stable")
    sidx = idxs_np[order]
    estart = np.searchsorted(sidx, np.arange(N_EXP + 1))
    erows = [order[estart[e]:estart[e + 1]] for e in range(N_EXP)]

    if counts.max() <= MAX_BS:
        # Expert-sharded path: rank experts by count; ordinal o takes the
        # experts ranked [8o, 8o+8), one per core, with a shared block
        # size that covers the largest of them.
        rank = np.argsort(-counts, kind="stable")
        gmax = [
            int(counts[rank[o * N_CORES:(o + 1) * N_CORES]].max())
            for o in range(EPC)
        ]
        # ordinals descending in size; put the smallest block FIRST (its
        # batch is the first DMA the PE waits on) and the second-smallest
        # LAST (short PE/relu/writeback tail after the stream ends)
        perm = [EPC - 1] + list(range(EPC - 2)) + [EPC - 2]
        rank = np.concatenate(
            [rank[o * N_CORES:(o + 1) * N_CORES] for o in perm]
        )
        BS = [max(16, -(-gmax[o] // 16) * 16) for o in perm]
        offs = np.concatenate([[0], np.cumsum(BS)])
        T = int(offs[-1])
        nc = _build_program(BS)

        in_maps = []
        core_blocks = []
        for c in range(N_CORES):
            blocks = [
                (int(rank[o * N_CORES + c]), erows[rank[o * N_CORES + c]])
                for o in range(EPC)
            ]
            core_blocks.append(blocks)
            in_maps.append(_pack_core(x, wprep, bprep, blocks, BS, offs, T))

        res = _run(nc, in_maps, trace)
        for c in range(N_CORES):
            _unpack_core(y, res.results[c]["y"], core_blocks[c], offs, T)
        return y

    # Fallback for pathological skew: sorted-shard into 128-sample
    # single-expert segments (one weight slot per segment), multi-pass.
    S = B // N_CORES
    per_core = []
    for c in range(N_CORES):
        e = sidx[c * S:(c + 1) * S]
        sel = order[c * S:(c + 1) * S]
        segs = []
        i = 0
        while i < S:
            j = i
            while j < S and e[j] == e[i]:
                j += 1
            k = i
            while k < j:
                cnt = min(SEG, j - k)
                segs.append((int(e[i]), sel[k:k + cnt]))
                k += cnt
            i = j
        per_core.append(segs)

    npass = max(1, (max(len(s) for s in per_core) + MAX_NSEG - 1) // MAX_NSEG)
    nseg = MAX_NSEG if npass > 1 else max(2, max(len(s) for s in per_core))
    BS = [SEG] * nseg
    offs = np.concatenate([[0], np.cumsum(BS)])
    T = int(offs[-1])
    nc = _build_program(BS)

    for pi in range(npass):
        in_maps = []
        pass_blocks = []
        for c in range(N_CORES):
            segs = per_core[c][pi * MAX_NSEG:(pi + 1) * MAX_NSEG]
            blocks = [
                segs[s] if s < len(segs) else (0, order[0:0])
                for s in range(nseg)
            ]
            pass_blocks.append(blocks)
            in_maps.append(_pack_core(x, wprep, bprep, blocks, BS, offs, T))
        res = _run(nc, in_maps, trace)
        for c in range(N_CORES):
            _unpack_core(y, res.results[c]["y"], pass_blocks[c], offs, T)
    return y
